# revision 33
# baseline (speedup 1.0000x reference)
"""Trainium2 Bass kernel for nn_FFEdgeCountingLayer (fuzzy-logic edge layer).

Forward value of the reference (straight-through hard Gumbel-softmax equals
the hard one-hot to ~1e-7):
  op_idx[o]  = argmax_p(op_logits[o,:] + gumbel(u_op[o,:]))      (0 -> T-norm)
  t[o,i]     = argmax_e(edge_logits[o,op_idx,i,:] + gumbel(u_edge))
  w[n,o,i]   = x[n,i] (identity) | 1-x[n,i] (complement) | tau[o] (no_edge)
  out[n,o]   = min_i w  for T-norm,  max_i w  for T-conorm
where tau[o] = 1 for T-norm else 0.

gumbel(u) = -log(-log(u)) is strictly increasing, so with logits constant
along the argmax axis (jnp.ones in setup_inputs) argmax(logits + gumbel(u))
== argmax(u): the device kernel compares u directly — no transcendentals,
and jax's first-max tie rule is reproduced exactly.  (If logits were ever
non-constant, keys fall back to logits + gumbel(u) in fp32 on the host;
never taken for this problem's generator.)

Distribution: out_features sharded 256 -> 8 cores x 32 (each output node is
independent); x replicated.  Host concatenates the per-core [1024, 32]
slices.

Per-core program:
  coefficients a[o,i] in {-1,0,1}, b[o,i] in {0,1} give w = a*x + b; the
  per-o sign sig = +1/-1 (T-norm/T-conorm) folds max into min:
      out[n,o] = sig[o] * min_i( (sig*a)[o,i]*x[n,i] + (sig*b)[o,i] )
  Layout partitions = i, free = n: the affine is ONE tensor_scalar (or
  ScalarE activation) per (o, i-tile) with per-partition scalars; the i
  reduction = 3 tensor_tensor mins + PE transpose + free-axis min-reduce.
"""

import contextlib
import os
import sys

import numpy as np

for _p in ("/opt/trn_rl_repo",):
    if _p not in sys.path and os.path.isdir(_p):
        sys.path.insert(0, _p)

import concourse.bacc as bacc
from concourse import masks, mybir, tile
from concourse.bass_utils import run_bass_kernel_spmd

F32 = mybir.dt.float32
AF = mybir.ActivationFunctionType
OP = mybir.AluOpType

N_CORES = 8
N, I, O = 1024, 512, 256
OC = O // N_CORES  # 32 out-features per core
K = I // 128       # 4 i-tiles
J = N // 128       # 8 n-tiles

# engine split for the hot loop, from measured HW rates (ACT affine 729ns,
# Pool TS ~800ns, DVE TS 350 / TT 529 / reduce 613 per [128,1024] op):
# VectorE keeps only mins + reduce (the bottleneck chain); the four affines
# go 3x ScalarE + 1x GpSimd.  All three are bit-exact here: a in {0,+-1}
# makes the product exact, so FMA == mul-then-add rounding.
TS_ENGINE = ("act", "pool", "act", "pool")  # per-k engine, o >= 2
PHASE_B_REPEAT = 1  # >1 only for steady-state HW timing builds


def _body(tc, timing_mode=False):
    """timing_mode: replace the input DMAs with on-chip memsets (identical
    instruction stream otherwise) so steady-state HW timing isn't hidden
    behind the axon per-call input transfer."""
    nc = tc.nc
    if timing_mode:
        x_d = nc.dram_tensor("x", [N, I], F32, kind="Internal").ap()
        ek_d = nc.dram_tensor("ekeys", [OC, 2, I, 3], F32, kind="Internal").ap()
        ok_d = nc.dram_tensor("okeys", [OC, 2], F32, kind="Internal").ap()
        seed = nc.dram_tensor("seed_in", [8, 4], F32, kind="ExternalInput").ap()
    else:
        x_d = nc.dram_tensor("x", [N, I], F32, kind="ExternalInput").ap()
        ek_d = nc.dram_tensor("ekeys", [OC, 2, I, 3], F32,
                              kind="ExternalInput").ap()
        ok_d = nc.dram_tensor("okeys", [OC, 2], F32, kind="ExternalInput").ap()
    out_d = nc.dram_tensor("out", [N, OC], F32, kind="ExternalOutput").ap()

    with contextlib.ExitStack() as ctx:
        cpool = ctx.enter_context(tc.tile_pool(name="const", bufs=1))
        apool = ctx.enter_context(tc.tile_pool(name="phase_a", bufs=1))
        xpool = ctx.enter_context(tc.tile_pool(name="xload", bufs=2))
        wpool = ctx.enter_context(tc.tile_pool(name="w", bufs=6))
        mpool = ctx.enter_context(tc.tile_pool(name="m", bufs=4))
        # single PSUM tag: [128, 2048] tiles (4 banks) x 2 bufs = all 8 banks
        pspool = ctx.enter_context(tc.tile_pool(name="ps", bufs=2, space="PSUM"))

        ident = cpool.tile([128, 128], F32, tag="ident")
        masks.make_identity(nc, ident[:])

        # ---- phase A: selections -> per-(o,i) coefficients ----
        # Partition row = k*OC + o (k = i-quarter), free = (p, i_sub, e): full
        # 128-partition utilization, all ops partition-aligned, and one
        # [128,128] PE transpose yields exactly the phase-B coefficient
        # layout acT[i_sub, k*OC + o].  These DMAs are issued FIRST so phase A
        # isn't queued behind the 2 MiB x load.
        ue = apool.tile([128, 2, 128, 3], F32, tag="ue")
        ok4 = apool.tile([128, 2], F32, tag="ok4")
        for k in range(K):
            nc.sync.dma_start(ok4[k * OC:(k + 1) * OC], ok_d[:])
        for k in range(K):
            nc.sync.dma_start(
                ue[k * OC:(k + 1) * OC],
                ek_d[:, :, k * 128:(k + 1) * 128, :],
            )

        tau = cpool.tile([128, 1], F32, tag="tau")   # tau[k*OC+o] = tau[o]
        sig = cpool.tile([128, 1], F32, tag="sig")
        nc.vector.tensor_tensor(tau[:], ok4[:, 0:1], ok4[:, 1:2], op=OP.is_ge)
        nc.vector.tensor_scalar(sig[:], tau[:], 2.0, -1.0, op0=OP.mult, op1=OP.add)
        # row-form sign broadcast to all partitions: sig_b[128, OC]
        ps_sig = pspool.tile([128, 2048], F32, tag="ps2048", name="ps_sig")
        nc.tensor.transpose(ps_sig[0:1, 0:OC], sig[0:OC], ident[0:OC, 0:OC])
        sig_row = cpool.tile([1, OC], F32, tag="sigrow")
        nc.scalar.copy(sig_row[:], ps_sig[0:1, 0:OC])
        sig_b = cpool.tile([128, OC], F32, tag="sig_b")
        nc.gpsimd.partition_broadcast(sig_b[:], sig_row[:])

        u0, u1, u2 = ue[:, :, :, 0], ue[:, :, :, 1], ue[:, :, :, 2]
        c01 = apool.tile([128, 2, 128], F32, tag="c01")
        c02 = apool.tile([128, 2, 128], F32, tag="c02")
        c12 = apool.tile([128, 2, 128], F32, tag="c12")
        nc.vector.tensor_tensor(c01[:], u0, u1, op=OP.is_ge)
        nc.vector.tensor_tensor(c02[:], u0, u2, op=OP.is_ge)
        nc.vector.tensor_tensor(c12[:], u1, u2, op=OP.is_ge)
        m0 = apool.tile([128, 2, 128], F32, tag="m0")
        m1 = apool.tile([128, 2, 128], F32, tag="m1")
        m2 = apool.tile([128, 2, 128], F32, tag="m2")
        nc.vector.tensor_tensor(m0[:], c01[:], c02[:], op=OP.mult)
        nc.vector.tensor_tensor(m1[:], c12[:], c01[:], op=OP.mult)
        nc.vector.tensor_tensor(m1[:], c12[:], m1[:], op=OP.subtract)
        nc.vector.tensor_tensor(m2[:], m0[:], m1[:], op=OP.add)
        nc.scalar.activation(m2[:], m2[:], AF.Identity, bias=1.0, scale=-1.0)

        a2 = apool.tile([128, 2, 128], F32, tag="a2")
        b2 = apool.tile([128, 2, 128], F32, tag="b2")
        nc.vector.tensor_tensor(a2[:], m0[:], m1[:], op=OP.subtract)
        nc.scalar.activation(b2[:], m2[:], AF.Identity, bias=0.0, scale=tau[:])
        nc.vector.tensor_tensor(b2[:], m1[:], b2[:], op=OP.add)
        nc.vector.tensor_scalar(a2[:], a2[:], sig[:], None, op0=OP.mult)
        nc.vector.tensor_scalar(b2[:], b2[:], sig[:], None, op0=OP.mult)

        # select p* slab (free-dim offsets): f = tau*(p0 - p1) + p1
        af = apool.tile([128, 128], F32, tag="af")
        bf = apool.tile([128, 128], F32, tag="bf")
        for src, dst in ((a2, af), (b2, bf)):
            nc.vector.tensor_tensor(dst[:], src[:, 0], src[:, 1], op=OP.subtract)
            nc.vector.tensor_scalar(dst[:], dst[:], tau[:], None, op0=OP.mult)
            nc.vector.tensor_tensor(dst[:], dst[:], src[:, 1], op=OP.add)

        # one PE transpose each -> acT[i_sub, k*OC + o]
        acT = cpool.tile([128, K * OC], F32, tag="acT")
        bcT = cpool.tile([128, K * OC], F32, tag="bcT")
        ps_ab = pspool.tile([128, 2048], F32, tag="ps2048", name="ps_ab")
        for i, (src, dst) in enumerate(((af, acT), (bf, bcT))):
            half = ps_ab[:, i * 1024:i * 1024 + K * OC]
            nc.tensor.transpose(half, src[:], ident[:])
            nc.scalar.copy(dst[:], half)

        # ---- load x[n,i] and PE-transpose to xT_k[i_sub=128, n=1024] ----
        # one column-block DMA per k so xT[k] is gated only by its own 512 KiB
        xT = [cpool.tile([128, N], F32, tag=f"xT{k}", name=f"xT{k}")
              for k in range(K)]
        x_v = x_d.rearrange("(j np) (k i) -> np j k i", np=128, k=K)
        for kp in range(K // 2):
            ps = pspool.tile([128, 2048], F32, tag="ps2048", name=f"ps_x{kp}")
            for kk in range(2):
                k = kp * 2 + kk
                xk = xpool.tile([128, J, 128], F32, tag="xk", name=f"xk{k}")
                nc.sync.dma_start(xk[:], x_v[:, :, k, :])
                for j in range(J):
                    nc.tensor.transpose(
                        ps[:, kk * N + j * 128:kk * N + (j + 1) * 128],
                        xk[:, j, :],
                        ident[:],
                    )
                nc.scalar.copy(xT[k][:], ps[:, kk * N:(kk + 1) * N])

        # ---- phase B: per-o affine + min over i ----
        red = cpool.tile([128, J * OC], F32, tag="red")    # col = j*OC + o
        outt = cpool.tile([128, J * OC], F32, tag="outt")
        red_v = red[:].rearrange("p (j o) -> p j o", o=OC)
        for op_ in [p for _ in range(PHASE_B_REPEAT) for p in range(OC // 2)]:
            # process an o-pair per psum tile: one FD-2048 reduce per pair
            maccs = []
            for oo in range(2):
                o = op_ * 2 + oo
                # paired tiles so the first tree level is ONE wide TT-min:
                # wa holds (k=0, k=2), wb holds (k=1, k=3)
                wa = wpool.tile([128, 2, N], F32, tag="wa")
                wb = wpool.tile([128, 2, N], F32, tag="wb")
                for k in range(K):
                    dst = (wa if k % 2 == 0 else wb)[:, k // 2, :]
                    col = k * OC + o
                    # first-pair warmup on Pool: it idles through phase A,
                    # and this keeps VectorE (the bottleneck) min-only
                    eng = "pool" if o < 2 else TS_ENGINE[k]
                    if eng == "act":
                        nc.scalar.activation(
                            dst, xT[k][:], AF.Identity,
                            bias=bcT[:, col:col + 1], scale=acT[:, col:col + 1],
                        )
                    elif eng == "pool":
                        nc.gpsimd.tensor_scalar(
                            dst, xT[k][:],
                            acT[:, col:col + 1], bcT[:, col:col + 1],
                            op0=OP.mult, op1=OP.add,
                        )
                    else:
                        nc.vector.tensor_scalar(
                            dst, xT[k][:],
                            acT[:, col:col + 1], bcT[:, col:col + 1],
                            op0=OP.mult, op1=OP.add,
                        )
                mab = mpool.tile([128, 2, N], F32, tag="mab")
                macc = mpool.tile([128, N], F32, tag="macc")
                nc.vector.tensor_tensor(mab[:], wa[:], wb[:], op=OP.min)
                nc.vector.tensor_tensor(macc[:], mab[:, 0, :], mab[:, 1, :],
                                        op=OP.min)
                maccs.append(macc)

            ps = pspool.tile([128, 2048], F32, tag="ps2048")
            for j in range(J):
                for oo in range(2):
                    nc.tensor.transpose(
                        ps[:, (j * 2 + oo) * 128:(j * 2 + oo + 1) * 128],
                        maccs[oo][:, j * 128:(j + 1) * 128],
                        ident[:],
                    )
            o0 = op_ * 2
            nc.vector.tensor_reduce(
                red_v[:, :, o0:o0 + 2],
                ps[:].rearrange("p (j oo i) -> p j oo i", oo=2, i=128),
                axis=mybir.AxisListType.X,
                op=OP.min,
            )

        for j in range(J):
            nc.vector.tensor_tensor(
                outt[:, j * OC:(j + 1) * OC],
                red[:, j * OC:(j + 1) * OC],
                sig_b[:],
                op=OP.mult,
            )
            nc.sync.dma_start(
                out_d[j * 128:(j + 1) * 128, :],
                outt[:, j * OC:(j + 1) * OC],
            )


_NC_CACHE = {}


def _build(repeat=1, timing_mode=False):
    key = f"nc_{repeat}_{timing_mode}"
    if key not in _NC_CACHE:
        global PHASE_B_REPEAT
        prev, PHASE_B_REPEAT = PHASE_B_REPEAT, repeat
        try:
            nc = bacc.Bacc("TRN2", target_bir_lowering=False, debug=False)
            with tile.TileContext(nc) as tc:
                _body(tc, timing_mode=timing_mode)
            nc.compile()
        finally:
            PHASE_B_REPEAT = prev
        _NC_CACHE[key] = nc
    return _NC_CACHE[key]


def _keys(logits, u):
    """Comparison keys whose argmax equals argmax(logits + gumbel(u))."""
    if np.all(logits == logits[..., :1]):
        return u
    return (logits + -np.log(-np.log(u))).astype(np.float32)


def kernel(x, edge_logits, op_logits, u_edge, u_op):
    x = np.ascontiguousarray(np.asarray(x, np.float32))
    ek = _keys(np.asarray(edge_logits, np.float32),
               np.ascontiguousarray(np.asarray(u_edge, np.float32)))
    ok = _keys(np.asarray(op_logits, np.float32),
               np.ascontiguousarray(np.asarray(u_op, np.float32)))

    nc = _build()
    in_maps = [
        {
            "x": x,
            "ekeys": np.ascontiguousarray(ek[c * OC:(c + 1) * OC]),
            "okeys": np.ascontiguousarray(ok[c * OC:(c + 1) * OC]),
        }
        for c in range(N_CORES)
    ]
    res = run_bass_kernel_spmd(nc, in_maps, core_ids=list(range(N_CORES)))
    _NC_CACHE["last_results"] = res
    out = np.concatenate([res.results[c]["out"] for c in range(N_CORES)], axis=1)
    return out.astype(np.float32)
